# revision 5
# baseline (speedup 1.0000x reference)
"""ConditionalPointFFM fused Bass kernel for 8 TRN2 NeuronCores.

Model (B=4 batches, N=32768 query points, M=512 obs points):
  point_feat = MLP_p(concat[coords, x_t, t])                  [B,N,256]
  obs_feat   = MLP_o(concat[obs_coords, obs_values, fe]) * mask
  d2         = |q - o|^2 (masked)  ->  w = softmax(-d2/(2s^2))
  local_cond = w @ obs_feat                                   [B,N,128]
  g          = MLP_g(point_feat.mean(axis=1))
  out        = MLP_h(concat[point_feat, g, local_cond])       [B,N,4]

Sharding: every core holds all 4 batches and a contiguous 4096-point
slice of N. Obs tensors + params replicated. The mean over N needs an
all-reduce; device collectives are unavailable here, so we run two
launches and combine the 8 per-core partial sums on the host between
them (the tiny global MLP still runs on device in launch 2).

Numerics: MLP matmuls run as float32r (full PE rate, ~1.7e-4); the d2
matmul runs true fp32 (f32r's rounding is amplified x200 by the RBF
exp). exp(-200*d2) needs no max-subtraction: d2min <= 0.435 with
overwhelming probability, so the unnormalized weights stay in fp32
range; the denominator comes from a ones-matrix matmul which directly
yields a partition-broadcast sum.

Layout: all activations feature-major [feat(partition), points(free)],
so y = x @ W becomes matmul(lhsT=W, rhs=x_fm) and the per-feature bias
is the ACT instruction's per-partition bias operand. d2 is produced
obs-major by a single K=5 matmul: lhsT rows [-2*o, oo + (1-mask)*1e6, 1],
rhs rows [q, 1, qq].
"""

import numpy as np

import concourse.bacc as bacc
import concourse.tile as tile
import concourse.mybir as mybir
import concourse.bass_isa as bass_isa
from concourse.tile_rust import add_dep_helper
from concourse import bass_utils

N_CORES = 8
B = 4
NPTS = 32768
PPC = NPTS // N_CORES          # points per core per batch
CH = 512                       # chunk of points processed at once
NCH = PPC // CH                # chunks per batch per core
NOBS = 512
HID = 256
COND = 128
FED = 32
NF = 4
SCALE = float(np.float32(-1.0 / (2 * 0.05 ** 2 + 1e-12)))   # == -200.0

F32 = mybir.dt.float32
F32R = mybir.dt.float32r
AF = mybir.ActivationFunctionType


def _load_w(nc, pool, name, dram, kin, kout):
    """DRAM weight [kin, kout] -> sbuf tile [128, (kin/128)*kout] in k-chunks,
    f32r. Returns (tile, slicer) where slicer(k, m) gives the [128,128] (or
    [kin,128]) lhsT AP for contraction chunk k, output chunk m."""
    nkin = max(1, kin // 128)
    pdim = min(kin, 128)
    t = pool.tile([pdim, nkin * kout], F32R, tag=name)
    for k in range(nkin):
        nc.sync.dma_start(t[:, k * kout:(k + 1) * kout],
                          dram.ap()[k * 128:k * 128 + pdim, :])

    def sl(k, m):
        return t[:, k * kout + m * 128: k * kout + m * 128 + 128]
    return t, sl


def _load_b(nc, pool, name, dram, n):
    """DRAM bias [n] -> sbuf [128, n/128] fp32 column tiles."""
    ncol = max(1, n // 128)
    pdim = min(n, 128)
    t = pool.tile([pdim, ncol], F32, tag=name)
    for m in range(ncol):
        nc.sync.dma_start(t[:, m:m + 1], dram.ap()[m * 128:m * 128 + pdim])
    return t


def build_launch1(rep=1):
    """Point-encoder partial sums: out acc [128, 8] (feat, b*2+ftile)."""
    nc = bacc.Bacc("TRN2", target_bir_lowering=False, debug=False,
                   num_devices=N_CORES)
    xin = nc.dram_tensor("xin", [B, 8, PPC], F32R, kind="ExternalInput")
    wp1 = nc.dram_tensor("wp1", [8, HID], F32R, kind="ExternalInput")
    wp2 = nc.dram_tensor("wp2", [HID, HID], F32R, kind="ExternalInput")
    wp3 = nc.dram_tensor("wp3", [HID, HID], F32R, kind="ExternalInput")
    bp1 = nc.dram_tensor("bp1", [HID], F32, kind="ExternalInput")
    bp2 = nc.dram_tensor("bp2", [HID], F32, kind="ExternalInput")
    bp3 = nc.dram_tensor("bp3", [HID], F32, kind="ExternalInput")
    acc_out = nc.dram_tensor("acc", [128, 2 * B], F32, kind="ExternalOutput")

    with tile.TileContext(nc) as tc:
        with (
            tc.tile_pool(name="const", bufs=1) as cp,
            tc.tile_pool(name="work", bufs=3) as wp,
            tc.tile_pool(name="ps", bufs=4, space="PSUM") as pp,
        ):
            _, wp1s = _load_w(nc, cp, "wp1", wp1, 8, HID)
            _, wp2s = _load_w(nc, cp, "wp2", wp2, HID, HID)
            _, wp3s = _load_w(nc, cp, "wp3", wp3, HID, HID)
            bp1t = _load_b(nc, cp, "bp1", bp1, HID)
            bp2t = _load_b(nc, cp, "bp2", bp2, HID)
            bp3t = _load_b(nc, cp, "bp3", bp3, HID)
            acc = cp.tile([128, 2 * B], F32, tag="accsb")
            nc.vector.memset(acc[:], 0.0)

            for _ in range(rep):
                for b in range(B):
                    for s in range(NCH):
                        sl = slice(s * CH, (s + 1) * CH)
                        infm = wp.tile([8, CH], F32R, tag="infm")
                        nc.sync.dma_start(infm[:], xin.ap()[b, :, sl])
                        h1 = [wp.tile([128, CH], F32R, tag=f"h1{m}", name=f"h1{m}")
                              for m in range(2)]
                        for m in range(2):
                            ps = pp.tile([128, CH], F32, tag="mlp")
                            nc.tensor.matmul(ps[:], wp1s(0, m), infm[:],
                                             start=True, stop=True)
                            nc.scalar.activation(h1[m][:], ps[:], AF.Gelu,
                                                 bias=bp1t[:, m:m + 1])
                        h2 = [wp.tile([128, CH], F32R, tag=f"h2{m}", name=f"h2{m}")
                              for m in range(2)]
                        for m in range(2):
                            ps = pp.tile([128, CH], F32, tag="mlp")
                            for k in range(2):
                                nc.tensor.matmul(ps[:], wp2s(k, m), h1[k][:],
                                                 start=(k == 0), stop=(k == 1))
                            nc.scalar.activation(h2[m][:], ps[:], AF.Gelu,
                                                 bias=bp2t[:, m:m + 1])
                        for m in range(2):
                            ps = pp.tile([128, CH], F32, tag="mlp")
                            for k in range(2):
                                nc.tensor.matmul(ps[:], wp3s(k, m), h2[k][:],
                                                 start=(k == 0), stop=(k == 1))
                            pf = wp.tile([128, CH], F32, tag="pf")
                            red = wp.tile([128, 1], F32, tag="red")
                            nc.scalar.activation(pf[:], ps[:], AF.Identity,
                                                 bias=bp3t[:, m:m + 1],
                                                 accum_out=red[:])
                            col = b * 2 + m
                            nc.vector.tensor_add(out=acc[:, col:col + 1],
                                                 in0=acc[:, col:col + 1],
                                                 in1=red[:])
            nc.sync.dma_start(acc_out.ap(), acc[:])
    nc.compile()
    return nc


def build_launch2(rep=1):
    nc = bacc.Bacc("TRN2", target_bir_lowering=False, debug=False,
                   num_devices=N_CORES)
    dt_in = [
        ("xin", [B, 8, PPC], F32R), ("cq", [B, 4, PPC], F32),
        ("obs_lhst", [B, 5, NOBS], F32), ("obs_in", [B, 36, NOBS], F32R),
        ("obs_mask", [B, NOBS], F32), ("mean_in", [2, 128, B], F32R),
        ("wp1", [8, HID], F32R), ("wp2", [HID, HID], F32R),
        ("wp3", [HID, HID], F32R),
        ("bp1", [HID], F32), ("bp2", [HID], F32), ("bp3", [HID], F32),
        ("wo1", [36, COND], F32R), ("wo2", [COND, COND], F32R),
        ("wo3", [COND, COND], F32R),
        ("bo1", [COND], F32), ("bo2", [COND], F32), ("bo3r", [1, COND], F32R),
        ("wg1", [HID, HID], F32R), ("wg2", [HID, HID], F32R),
        ("bg1", [HID], F32), ("bg2", [HID], F32),
        ("wh1pf", [HID, HID], F32R), ("wh1g", [HID, HID], F32R),
        ("wh1lc", [COND, HID], F32R), ("bh1", [HID], F32),
        ("wh2", [HID, HID], F32R), ("bh2", [HID], F32),
        ("wh3", [HID, NF], F32R), ("bh3", [NF], F32),
        ("ones1", [1, 128], F32R), ("ones128", [128, 128], F32R),
    ]
    d = {name: nc.dram_tensor(name, shp, dt, kind="ExternalInput")
         for name, shp, dt in dt_in}
    y = nc.dram_tensor("y", [B, NF, PPC], F32, kind="ExternalOutput")

    acts = []          # ACT instructions in intended engine order, per pass
    cur_pass = []

    def end_pass():
        nonlocal cur_pass
        if cur_pass:
            acts.append(cur_pass)
            cur_pass = []

    def act(*args, **kw):
        inst = nc.scalar.activation(*args, **kw)
        cur_pass.append(inst)
        return inst

    with tile.TileContext(nc) as tc:
        with (
            tc.tile_pool(name="const", bufs=1) as cp,
            tc.tile_pool(name="big", bufs=1) as bigp,
            tc.tile_pool(name="work", bufs=3) as wkp,
            tc.tile_pool(name="ps_mlp", bufs=4, space="PSUM") as pmlp,
            tc.tile_pool(name="ps_d2", bufs=2, space="PSUM") as pd2,
            tc.tile_pool(name="ps_agg", bufs=1, space="PSUM") as pagg,
        ):
            # ---- constants ----
            _, wp1s = _load_w(nc, cp, "wp1", d["wp1"], 8, HID)
            _, wp2s = _load_w(nc, cp, "wp2", d["wp2"], HID, HID)
            _, wp3s = _load_w(nc, cp, "wp3", d["wp3"], HID, HID)
            bp1t = _load_b(nc, cp, "bp1", d["bp1"], HID)
            bp2t = _load_b(nc, cp, "bp2", d["bp2"], HID)
            bp3t = _load_b(nc, cp, "bp3", d["bp3"], HID)
            wo1t, _ = _load_w(nc, cp, "wo1", d["wo1"], 36, COND)
            wo2t, _ = _load_w(nc, cp, "wo2", d["wo2"], COND, COND)
            wo3t, _ = _load_w(nc, cp, "wo3", d["wo3"], COND, COND)
            bo1t = _load_b(nc, cp, "bo1", d["bo1"], COND)
            bo2t = _load_b(nc, cp, "bo2", d["bo2"], COND)
            bo3r = cp.tile([1, COND], F32R, tag="bo3r")
            nc.sync.dma_start(bo3r[:], d["bo3r"].ap())
            _, wg1s = _load_w(nc, cp, "wg1", d["wg1"], HID, HID)
            _, wg2s = _load_w(nc, cp, "wg2", d["wg2"], HID, HID)
            bg1t = _load_b(nc, cp, "bg1", d["bg1"], HID)
            bg2t = _load_b(nc, cp, "bg2", d["bg2"], HID)
            _, wh1pfs = _load_w(nc, cp, "wh1pf", d["wh1pf"], HID, HID)
            _, wh1gs = _load_w(nc, cp, "wh1g", d["wh1g"], HID, HID)
            _, wh1lcs = _load_w(nc, cp, "wh1lc", d["wh1lc"], COND, HID)
            bh1t = _load_b(nc, cp, "bh1", d["bh1"], HID)
            _, wh2s = _load_w(nc, cp, "wh2", d["wh2"], HID, HID)
            bh2t = _load_b(nc, cp, "bh2", d["bh2"], HID)
            wh3t, _ = _load_w(nc, cp, "wh3", d["wh3"], HID, NF)
            bh3t = _load_b(nc, cp, "bh3", d["bh3"], NF)

            ones1 = cp.tile([1, 128], F32R, tag="ones1")
            nc.sync.dma_start(ones1[:], d["ones1"].ap())
            ones128 = cp.tile([128, 128], F32R, tag="ones128")
            nc.sync.dma_start(ones128[:], d["ones128"].ap())

            obs_lhst_t = {}
            for b in range(B):
                t = cp.tile([5, NOBS], F32, tag=f"obslhst{b}")
                nc.sync.dma_start(t[:], d["obs_lhst"].ap()[b])
                obs_lhst_t[b] = t
            mask_t = cp.tile([128, 4 * B], F32, tag="maskt")
            for b in range(B):
                for j in range(4):
                    nc.sync.dma_start(
                        mask_t[:, b * 4 + j: b * 4 + j + 1],
                        d["obs_mask"].ap()[b, j * 128:(j + 1) * 128])

            # ---- obs encoder (all batches), feature-major then transposed L3
            obsf = {}
            for b in range(B):
                oin = wkp.tile([36, NOBS], F32R, tag="oin")
                nc.sync.dma_start(oin[:], d["obs_in"].ap()[b])
                ps = pmlp.tile([128, NOBS], F32, tag="mlp")
                nc.tensor.matmul(ps[:], wo1t[:], oin[:], start=True, stop=True)
                of1 = wkp.tile([128, NOBS], F32R, tag="of1")
                act(of1[:], ps[:], AF.Gelu, bias=bo1t[:, 0:1])
                ps = pmlp.tile([128, NOBS], F32, tag="mlp")
                nc.tensor.matmul(ps[:], wo2t[:], of1[:], start=True, stop=True)
                of2 = wkp.tile([128, NOBS], F32R, tag="of2")
                act(of2[:], ps[:], AF.Gelu, bias=bo2t[:, 0:1])
                # L3 with transposed output: out[obs_chunk, cond]
                for j in range(4):
                    pst = pmlp.tile([128, COND], F32, tag="mlp")
                    nc.tensor.matmul(pst[:], of2[:, j * 128:(j + 1) * 128],
                                     wo3t[:], start=True, stop=False)
                    nc.tensor.matmul(pst[:], ones1[:], bo3r[:],
                                     start=False, stop=True)
                    oft = cp.tile([128, COND], F32R, tag=f"obsf{b}_{j}")
                    nc.vector.tensor_scalar_mul(
                        oft[:], pst[:], mask_t[:, b * 4 + j: b * 4 + j + 1])
                    obsf[(b, j)] = oft

            # ---- global feature MLP + head-bias fold (tiny) ----
            mean_t = []
            for k in range(2):
                mt = cp.tile([128, B], F32R, tag=f"mean{k}")
                nc.sync.dma_start(mt[:], d["mean_in"].ap()[k])
                mean_t.append(mt)
            g1 = []
            for m in range(2):
                ps = pmlp.tile([128, B], F32, tag="mlp")
                for k in range(2):
                    nc.tensor.matmul(ps[:], wg1s(k, m), mean_t[k][:],
                                     start=(k == 0), stop=(k == 1))
                t = cp.tile([128, B], F32R, tag=f"g1{m}")
                act(t[:], ps[:], AF.Gelu, bias=bg1t[:, m:m + 1])
                g1.append(t)
            g2 = []
            for m in range(2):
                ps = pmlp.tile([128, B], F32, tag="mlp")
                for k in range(2):
                    nc.tensor.matmul(ps[:], wg2s(k, m), g1[k][:],
                                     start=(k == 0), stop=(k == 1))
                t = cp.tile([128, B], F32R, tag=f"g2{m}")
                act(t[:], ps[:], AF.Identity, bias=bg2t[:, m:m + 1])
                g2.append(t)
            gb = []
            for m in range(2):
                ps = pmlp.tile([128, B], F32, tag="mlp")
                for k in range(2):
                    nc.tensor.matmul(ps[:], wh1gs(k, m), g2[k][:],
                                     start=(k == 0), stop=(k == 1))
                t = cp.tile([128, B], F32, tag=f"gb{m}")
                act(t[:], ps[:], AF.Identity, bias=bh1t[:, m:m + 1])
                gb.append(t)
            end_pass()   # preamble: gelu set

            # ---- main per-batch loop: A (gelu) -> B (exp) -> C (gelu) ----
            for _ in range(rep):
                for b in range(B):
                    pfb = bigp.tile([128, 2 * PPC], F32R, tag="pfb")
                    lcb = bigp.tile([128, PPC], F32R, tag="lcb")

                    # pass A: point encoder
                    for s in range(NCH):
                        sl = slice(s * CH, (s + 1) * CH)
                        infm = wkp.tile([8, CH], F32R, tag="infm")
                        nc.sync.dma_start(infm[:], d["xin"].ap()[b, :, sl])
                        h1 = [wkp.tile([128, CH], F32R, tag=f"h1{m}", name=f"h1{m}")
                              for m in range(2)]
                        for m in range(2):
                            ps = pmlp.tile([128, CH], F32, tag="mlp")
                            nc.tensor.matmul(ps[:], wp1s(0, m), infm[:],
                                             start=True, stop=True)
                            act(h1[m][:], ps[:], AF.Gelu, bias=bp1t[:, m:m + 1])
                        h2 = [wkp.tile([128, CH], F32R, tag=f"h2{m}", name=f"h2{m}")
                              for m in range(2)]
                        for m in range(2):
                            ps = pmlp.tile([128, CH], F32, tag="mlp")
                            for k in range(2):
                                nc.tensor.matmul(ps[:], wp2s(k, m), h1[k][:],
                                                 start=(k == 0), stop=(k == 1))
                            act(h2[m][:], ps[:], AF.Gelu, bias=bp2t[:, m:m + 1])
                        for m in range(2):
                            ps = pmlp.tile([128, CH], F32, tag="mlp")
                            for k in range(2):
                                nc.tensor.matmul(ps[:], wp3s(k, m), h2[k][:],
                                                 start=(k == 0), stop=(k == 1))
                            act(pfb[:, m * PPC + s * CH: m * PPC + (s + 1) * CH],
                                ps[:], AF.Identity, bias=bp3t[:, m:m + 1])
                    end_pass()

                    # pass B: RBF softmax aggregation
                    for s in range(NCH):
                        sl = slice(s * CH, (s + 1) * CH)
                        rhs = wkp.tile([5, CH], F32, tag="rhsd2")
                        nc.sync.dma_start(rhs[0:4, :], d["cq"].ap()[b, :, sl])
                        sq = wkp.tile([3, CH], F32, tag="sq")
                        nc.vector.tensor_mul(out=sq[:], in0=rhs[0:3, :],
                                             in1=rhs[0:3, :])
                        q3 = wkp.tile([3, CH], F32, tag="q3")
                        nc.gpsimd.partition_all_reduce(
                            q3[:], sq[:], channels=3,
                            reduce_op=bass_isa.ReduceOp.add)
                        nc.sync.dma_start(rhs[4:5, :], q3[0:1, :])

                        s_ps = pagg.tile([128, CH], F32, tag="s_ps")
                        lc_ps = pagg.tile([128, CH], F32, tag="lc_ps")
                        for j in range(4):
                            dps = pd2.tile([128, CH], F32, tag="d2")
                            nc.tensor.matmul(
                                dps[:],
                                obs_lhst_t[b][:, j * 128:(j + 1) * 128],
                                rhs[:], start=True, stop=True)
                            e = wkp.tile([128, CH], F32R, tag="e")
                            act(e[:], dps[:], AF.Exp, scale=SCALE)
                            nc.tensor.matmul(s_ps[:], ones128[:], e[:],
                                             start=(j == 0), stop=(j == 3))
                            nc.tensor.matmul(lc_ps[:], obsf[(b, j)][:], e[:],
                                             start=(j == 0), stop=(j == 3))
                        rec = wkp.tile([128, CH], F32, tag="rec")
                        nc.vector.reciprocal(rec[:], s_ps[:])
                        nc.vector.tensor_mul(out=lcb[:, sl], in0=lc_ps[:],
                                             in1=rec[:])
                    end_pass()

                    # pass C: head
                    for s in range(NCH):
                        sl = slice(s * CH, (s + 1) * CH)
                        hc1 = [wkp.tile([128, CH], F32R, tag=f"hc1{m}", name=f"hc1{m}")
                               for m in range(2)]
                        for m in range(2):
                            ps = pmlp.tile([128, CH], F32, tag="mlp")
                            for k in range(2):
                                nc.tensor.matmul(
                                    ps[:], wh1pfs(k, m),
                                    pfb[:, k * PPC + s * CH: k * PPC + (s + 1) * CH],
                                    start=(k == 0), stop=False)
                            nc.tensor.matmul(ps[:], wh1lcs(0, m), lcb[:, sl],
                                             start=False, stop=True)
                            act(hc1[m][:], ps[:], AF.Gelu,
                                bias=gb[m][:, b:b + 1])
                        hc2 = [wkp.tile([128, CH], F32R, tag=f"hc2{m}", name=f"hc2{m}")
                               for m in range(2)]
                        for m in range(2):
                            ps = pmlp.tile([128, CH], F32, tag="mlp")
                            for k in range(2):
                                nc.tensor.matmul(ps[:], wh2s(k, m), hc1[k][:],
                                                 start=(k == 0), stop=(k == 1))
                            act(hc2[m][:], ps[:], AF.Gelu,
                                bias=bh2t[:, m:m + 1])
                        ps = pmlp.tile([NF, CH], F32, tag="mlp")
                        for k in range(2):
                            nc.tensor.matmul(ps[:], wh3t[:, k * NF:(k + 1) * NF],
                                             hc2[k][:],
                                             start=(k == 0), stop=(k == 1))
                        osb = wkp.tile([NF, CH], F32, tag="osb")
                        act(osb[:], ps[:], AF.Identity, bias=bh3t[:, 0:1])
                        nc.sync.dma_start(y.ap()[b, :, sl], osb[:])
                    end_pass()

            # enforce ACT engine-order across passes so walrus doesn't thrash
            # the activation table sets (gelu vs exp live in different sets)
            for i in range(1, len(acts)):
                add_dep_helper(acts[i][0].ins, acts[i - 1][-1].ins,
                               reason="ACT table-set ordering")
    nc.compile()
    return nc


# -------------------- host side --------------------

_CACHE = {}


def _get(name, rep=1):
    key = (name, rep)
    if key not in _CACHE:
        _CACHE[key] = (build_launch1(rep) if name == "l1" else
                       build_launch2(rep))
    return _CACHE[key]


def _prep(t, x_t, coords, obs_coords, obs_values, obs_mask, obs_field_ids,
          params):
    """Build the per-core input maps (host-side layout prep only)."""
    f32 = np.float32
    t = np.asarray(t, f32)
    x_t = np.asarray(x_t, f32)
    coords = np.asarray(coords, f32)
    obs_coords = np.asarray(obs_coords, f32)
    obs_values = np.asarray(obs_values, f32)
    obs_mask = np.asarray(obs_mask, f32)
    ids = np.asarray(obs_field_ids)

    coords_t = np.ascontiguousarray(coords.transpose(0, 2, 1))       # [B,3,N]
    xin = np.concatenate([
        coords_t,
        np.ascontiguousarray(x_t.transpose(0, 2, 1)),
        np.broadcast_to(t[:, None, None], (B, 1, NPTS)),
    ], axis=1).astype(f32)                                           # [B,8,N]

    fe = np.asarray(params["field_embed"], f32)[np.clip(ids, 0, None)]
    fe = fe * obs_mask[..., None]                                    # [B,M,32]
    obs_in = np.concatenate([obs_coords, obs_values, fe], axis=-1)   # [B,M,36]
    obs_in = np.ascontiguousarray(obs_in.transpose(0, 2, 1)).astype(f32)

    oo = (obs_coords * obs_coords).sum(-1).astype(f32)               # [B,M]
    obs_lhst = np.stack([
        -2.0 * obs_coords[:, :, 0], -2.0 * obs_coords[:, :, 1],
        -2.0 * obs_coords[:, :, 2],
        oo + (1.0 - obs_mask) * np.float32(1e6),
        np.ones((B, NOBS), f32),
    ], axis=1).astype(f32)                                           # [B,5,M]

    def wb(layers, i):
        w, bias = layers[i]
        return np.ascontiguousarray(np.asarray(w, f32)), \
            np.ascontiguousarray(np.asarray(bias, f32))

    pe = params["point_encoder"]
    oe = params["obs_encoder"]
    ge = params["global_encoder"]
    hd = params["head"]
    wp1, bp1 = wb(pe, 0); wp2, bp2 = wb(pe, 1); wp3, bp3 = wb(pe, 2)
    wo1, bo1 = wb(oe, 0); wo2, bo2 = wb(oe, 1); wo3, bo3 = wb(oe, 2)
    wg1, bg1 = wb(ge, 0); wg2, bg2 = wb(ge, 1)
    wh1, bh1 = wb(hd, 0); wh2, bh2 = wb(hd, 1); wh3, bh3 = wb(hd, 2)

    shared2 = dict(
        obs_in=obs_in, obs_lhst=obs_lhst, obs_mask=obs_mask,
        wp1=wp1, wp2=wp2, wp3=wp3, bp1=bp1, bp2=bp2, bp3=bp3,
        wo1=wo1, wo2=wo2, wo3=wo3, bo1=bo1, bo2=bo2,
        bo3r=np.ascontiguousarray(bo3[None, :]),
        wg1=wg1, wg2=wg2, bg1=bg1, bg2=bg2,
        wh1pf=np.ascontiguousarray(wh1[0:HID]),
        wh1g=np.ascontiguousarray(wh1[HID:2 * HID]),
        wh1lc=np.ascontiguousarray(wh1[2 * HID:]),
        bh1=bh1, wh2=wh2, bh2=bh2, wh3=wh3, bh3=bh3,
        ones1=np.ones((1, 128), np.float32),
        ones128=np.ones((128, 128), np.float32),
    )
    in1, in2 = [], []
    for c in range(N_CORES):
        sl = slice(c * PPC, (c + 1) * PPC)
        xin_c = np.ascontiguousarray(xin[:, :, sl])
        in1.append(dict(xin=xin_c, wp1=wp1, wp2=wp2, wp3=wp3,
                        bp1=bp1, bp2=bp2, bp3=bp3))
        m = dict(shared2)
        m["xin"] = xin_c
        m["cq"] = np.ascontiguousarray(
            np.concatenate([coords_t[:, :, sl],
                            np.ones((B, 1, PPC), np.float32)], axis=1))
        in2.append(m)
    return in1, in2


def _run_l1(in1, rep=1):
    nc = _get("l1", rep)
    res = bass_utils.run_bass_kernel_spmd(nc, in1,
                                          core_ids=list(range(N_CORES)))
    # acc [128, 2B] per core; combine in f64
    total = np.zeros((128, 2 * B), np.float64)
    for r in res.results:
        total += r["acc"].astype(np.float64)
    mean = total / float(NPTS)
    # mean_in [2, 128, B]: mean_in[k, f, b] = mean[f, b*2+k]
    mean_in = np.empty((2, 128, B), np.float32)
    for k in range(2):
        for b in range(B):
            mean_in[k, :, b] = mean[:, b * 2 + k].astype(np.float32)
    return mean_in


def _run_l2(in2, mean_in, rep=1):
    nc = _get("l2", rep)
    for m in in2:
        m["mean_in"] = mean_in
    res = bass_utils.run_bass_kernel_spmd(nc, in2,
                                          core_ids=list(range(N_CORES)))
    out = np.empty((B, NPTS, NF), np.float32)
    for c, r in enumerate(res.results):
        yc = r["y"]                                   # [B, NF, PPC]
        out[:, c * PPC:(c + 1) * PPC, :] = yc.transpose(0, 2, 1)
    return out


def kernel(t, x_t, coords, obs_coords, obs_values, obs_mask, obs_field_ids,
           params):
    in1, in2 = _prep(t, x_t, coords, obs_coords, obs_values, obs_mask,
                     obs_field_ids, params)
    mean_in = _run_l1(in1)
    return _run_l2(in2, mean_in)


# revision 7
# speedup vs baseline: 1.1438x; 1.1438x over previous
"""ConditionalPointFFM fused Bass kernel for 8 TRN2 NeuronCores.

Model (B=4 batches, N=32768 query points, M=512 obs points):
  point_feat = MLP_p(concat[coords, x_t, t])                  [B,N,256]
  obs_feat   = MLP_o(concat[obs_coords, obs_values, fe]) * mask
  d2         = |q - o|^2 (masked)  ->  w = softmax(-d2/(2s^2))
  local_cond = w @ obs_feat                                   [B,N,128]
  g          = MLP_g(point_feat.mean(axis=1))
  out        = MLP_h(concat[point_feat, g, local_cond])       [B,N,4]

Sharding: every core holds all 4 batches and a contiguous 4096-point
slice of N. Obs tensors + params replicated. The mean over N needs an
all-reduce; device collectives are unavailable here, so we run two
launches and combine the 8 per-core partial sums on the host between
them (the tiny global MLP still runs on device in launch 2).

Numerics: MLP matmuls run as float32r (full PE rate, ~1.7e-4); the d2
matmul runs true fp32 (f32r's rounding is amplified x200 by the RBF
exp). exp(-200*d2) needs no max-subtraction: d2min <= 0.435 with
overwhelming probability, so the unnormalized weights stay in fp32
range; the denominator comes from a ones-matrix matmul which directly
yields a partition-broadcast sum.

Layout: all activations feature-major [feat(partition), points(free)],
so y = x @ W becomes matmul(lhsT=W, rhs=x_fm) and the per-feature bias
is the ACT instruction's per-partition bias operand. d2 is produced
obs-major by a single K=5 matmul: lhsT rows [-2*o, oo + (1-mask)*1e6, 1],
rhs rows [q, 1, qq].
"""

import numpy as np

import concourse.bacc as bacc
import concourse.tile as tile
import concourse.mybir as mybir
import concourse.bass_isa as bass_isa
from concourse.tile_rust import add_dep_helper
from concourse import bass_utils

N_CORES = 8
B = 4
NPTS = 32768
PPC = NPTS // N_CORES          # points per core per batch
CH = 512                       # chunk of points processed at once
NCH = PPC // CH                # chunks per batch per core
NOBS = 512
HID = 256
COND = 128
FED = 32
NF = 4
SCALE = float(np.float32(-1.0 / (2 * 0.05 ** 2 + 1e-12)))   # == -200.0

F32 = mybir.dt.float32
F32R = mybir.dt.float32r
BF16 = mybir.dt.bfloat16
AF = mybir.ActivationFunctionType


def _load_w(nc, pool, name, dram, kin, kout):
    """DRAM weight [kin, kout] -> sbuf tile [128, (kin/128)*kout] in k-chunks,
    f32r. Returns (tile, slicer) where slicer(k, m) gives the [128,128] (or
    [kin,128]) lhsT AP for contraction chunk k, output chunk m."""
    nkin = max(1, kin // 128)
    pdim = min(kin, 128)
    t = pool.tile([pdim, nkin * kout], F32R, tag=name)
    for k in range(nkin):
        nc.sync.dma_start(t[:, k * kout:(k + 1) * kout],
                          dram.ap()[k * 128:k * 128 + pdim, :])

    def sl(k, m):
        return t[:, k * kout + m * 128: k * kout + m * 128 + 128]
    return t, sl


def _load_b(nc, pool, name, dram, n):
    """DRAM bias [n] -> sbuf [128, n/128] fp32 column tiles."""
    ncol = max(1, n // 128)
    pdim = min(n, 128)
    t = pool.tile([pdim, ncol], F32, tag=name)
    for m in range(ncol):
        nc.sync.dma_start(t[:, m:m + 1], dram.ap()[m * 128:m * 128 + pdim])
    return t


def build_launch1(rep=1):
    """Point-encoder partial sums: out acc [128, 8] (feat, b*2+ftile)."""
    nc = bacc.Bacc("TRN2", target_bir_lowering=False, debug=False,
                   num_devices=N_CORES)
    xin = nc.dram_tensor("xin", [B, 8, PPC], F32R, kind="ExternalInput")
    wp1 = nc.dram_tensor("wp1", [8, HID], F32R, kind="ExternalInput")
    wp2 = nc.dram_tensor("wp2", [HID, HID], F32R, kind="ExternalInput")
    wp3 = nc.dram_tensor("wp3", [HID, HID], F32R, kind="ExternalInput")
    bp1 = nc.dram_tensor("bp1", [HID], F32, kind="ExternalInput")
    bp2 = nc.dram_tensor("bp2", [HID], F32, kind="ExternalInput")
    bp3 = nc.dram_tensor("bp3", [HID], F32, kind="ExternalInput")
    acc_out = nc.dram_tensor("acc", [128, 2 * B], F32, kind="ExternalOutput")

    with tile.TileContext(nc) as tc:
        with (
            tc.tile_pool(name="const", bufs=1) as cp,
            tc.tile_pool(name="work", bufs=3) as wp,
            tc.tile_pool(name="ps", bufs=4, space="PSUM") as pp,
        ):
            _, wp1s = _load_w(nc, cp, "wp1", wp1, 8, HID)
            _, wp2s = _load_w(nc, cp, "wp2", wp2, HID, HID)
            _, wp3s = _load_w(nc, cp, "wp3", wp3, HID, HID)
            bp1t = _load_b(nc, cp, "bp1", bp1, HID)
            bp2t = _load_b(nc, cp, "bp2", bp2, HID)
            bp3t = _load_b(nc, cp, "bp3", bp3, HID)
            acc = cp.tile([128, 2 * B], F32, tag="accsb")
            nc.vector.memset(acc[:], 0.0)

            for _ in range(rep):
                for b in range(B):
                    for s in range(NCH):
                        sl = slice(s * CH, (s + 1) * CH)
                        infm = wp.tile([8, CH], F32R, tag="infm")
                        nc.sync.dma_start(infm[:], xin.ap()[b, :, sl])
                        h1 = [wp.tile([128, CH], F32R, tag=f"h1{m}", name=f"h1{m}")
                              for m in range(2)]
                        for m in range(2):
                            ps = pp.tile([128, CH], F32, tag="mlp")
                            nc.tensor.matmul(ps[:], wp1s(0, m), infm[:],
                                             start=True, stop=True)
                            nc.scalar.activation(h1[m][:], ps[:], AF.Gelu,
                                                 bias=bp1t[:, m:m + 1])
                        h2 = [wp.tile([128, CH], F32R, tag=f"h2{m}", name=f"h2{m}")
                              for m in range(2)]
                        for m in range(2):
                            ps = pp.tile([128, CH], F32, tag="mlp")
                            for k in range(2):
                                nc.tensor.matmul(ps[:], wp2s(k, m), h1[k][:],
                                                 start=(k == 0), stop=(k == 1))
                            nc.scalar.activation(h2[m][:], ps[:], AF.Gelu,
                                                 bias=bp2t[:, m:m + 1])
                        for m in range(2):
                            ps = pp.tile([128, CH], F32, tag="mlp")
                            for k in range(2):
                                nc.tensor.matmul(ps[:], wp3s(k, m), h2[k][:],
                                                 start=(k == 0), stop=(k == 1))
                            pf = wp.tile([128, CH], F32, tag="pf")
                            red = wp.tile([128, 1], F32, tag="red")
                            nc.scalar.activation(pf[:], ps[:], AF.Identity,
                                                 bias=bp3t[:, m:m + 1],
                                                 accum_out=red[:])
                            col = b * 2 + m
                            nc.vector.tensor_add(out=acc[:, col:col + 1],
                                                 in0=acc[:, col:col + 1],
                                                 in1=red[:])
            nc.sync.dma_start(acc_out.ap(), acc[:])
    nc.compile()
    return nc


def build_launch2(rep=1):
    nc = bacc.Bacc("TRN2", target_bir_lowering=False, debug=False,
                   num_devices=N_CORES)
    dt_in = [
        ("xin", [B, 8, PPC], F32R), ("cq_bf", [B, 22, PPC], BF16),
        ("obs_lhst", [B, 22, NOBS], BF16), ("obs_in", [B, 36, NOBS], F32R),
        ("obs_mask", [B, NOBS], F32), ("mean_in", [2, 128, B], F32R),
        ("wp1", [8, HID], F32R), ("wp2", [HID, HID], F32R),
        ("wp3", [HID, HID], F32R),
        ("bp1", [HID], F32), ("bp2", [HID], F32), ("bp3", [HID], F32),
        ("wo1", [36, COND], F32R), ("wo2", [COND, COND], F32R),
        ("wo3", [COND, COND], F32R),
        ("bo1", [COND], F32), ("bo2", [COND], F32), ("bo3r", [1, COND], F32R),
        ("wg1", [HID, HID], F32R), ("wg2", [HID, HID], F32R),
        ("bg1", [HID], F32), ("bg2", [HID], F32),
        ("wh1pf", [HID, HID], F32R), ("wh1g", [HID, HID], F32R),
        ("wh1lc", [COND, HID], F32R), ("bh1", [HID], F32),
        ("wh2", [HID, HID], F32R), ("bh2", [HID], F32),
        ("wh3", [HID, NF], F32R), ("bh3", [NF], F32),
        ("ones1", [1, 128], F32R), ("ones128", [128, 128], F32R),
    ]
    d = {name: nc.dram_tensor(name, shp, dt, kind="ExternalInput")
         for name, shp, dt in dt_in}
    y = nc.dram_tensor("y", [B, NF, PPC], F32, kind="ExternalOutput")

    acts = []          # ACT instructions in intended engine order, per pass
    cur_pass = []

    def end_pass():
        nonlocal cur_pass
        if cur_pass:
            acts.append(cur_pass)
            cur_pass = []

    def act(*args, **kw):
        inst = nc.scalar.activation(*args, **kw)
        cur_pass.append(inst)
        return inst

    with tile.TileContext(nc) as tc:
        with (
            tc.tile_pool(name="const", bufs=1) as cp,
            tc.tile_pool(name="big", bufs=1) as bigp,
            tc.tile_pool(name="work", bufs=3) as wkp,
            tc.tile_pool(name="ps_mlp", bufs=4, space="PSUM") as pmlp,
            tc.tile_pool(name="ps_d2", bufs=2, space="PSUM") as pd2,
            tc.tile_pool(name="ps_agg", bufs=1, space="PSUM") as pagg,
        ):
            # ---- constants ----
            _, wp1s = _load_w(nc, cp, "wp1", d["wp1"], 8, HID)
            _, wp2s = _load_w(nc, cp, "wp2", d["wp2"], HID, HID)
            _, wp3s = _load_w(nc, cp, "wp3", d["wp3"], HID, HID)
            bp1t = _load_b(nc, cp, "bp1", d["bp1"], HID)
            bp2t = _load_b(nc, cp, "bp2", d["bp2"], HID)
            bp3t = _load_b(nc, cp, "bp3", d["bp3"], HID)
            wo1t, _ = _load_w(nc, cp, "wo1", d["wo1"], 36, COND)
            wo2t, _ = _load_w(nc, cp, "wo2", d["wo2"], COND, COND)
            wo3t, _ = _load_w(nc, cp, "wo3", d["wo3"], COND, COND)
            bo1t = _load_b(nc, cp, "bo1", d["bo1"], COND)
            bo2t = _load_b(nc, cp, "bo2", d["bo2"], COND)
            bo3r = cp.tile([1, COND], F32R, tag="bo3r")
            nc.sync.dma_start(bo3r[:], d["bo3r"].ap())
            _, wg1s = _load_w(nc, cp, "wg1", d["wg1"], HID, HID)
            _, wg2s = _load_w(nc, cp, "wg2", d["wg2"], HID, HID)
            bg1t = _load_b(nc, cp, "bg1", d["bg1"], HID)
            bg2t = _load_b(nc, cp, "bg2", d["bg2"], HID)
            _, wh1pfs = _load_w(nc, cp, "wh1pf", d["wh1pf"], HID, HID)
            _, wh1gs = _load_w(nc, cp, "wh1g", d["wh1g"], HID, HID)
            _, wh1lcs = _load_w(nc, cp, "wh1lc", d["wh1lc"], COND, HID)
            bh1t = _load_b(nc, cp, "bh1", d["bh1"], HID)
            _, wh2s = _load_w(nc, cp, "wh2", d["wh2"], HID, HID)
            bh2t = _load_b(nc, cp, "bh2", d["bh2"], HID)
            wh3t, _ = _load_w(nc, cp, "wh3", d["wh3"], HID, NF)
            bh3t = _load_b(nc, cp, "bh3", d["bh3"], NF)

            ones1 = cp.tile([1, 128], F32R, tag="ones1")
            nc.sync.dma_start(ones1[:], d["ones1"].ap())
            ones128 = cp.tile([128, 128], F32R, tag="ones128")
            nc.sync.dma_start(ones128[:], d["ones128"].ap())

            obs_lhst_t = {}
            for b in range(B):
                t = cp.tile([22, NOBS], BF16, tag=f"obslhst{b}", name=f"obslhst{b}")
                nc.sync.dma_start(t[:], d["obs_lhst"].ap()[b])
                obs_lhst_t[b] = t
            mask_t = cp.tile([128, 4 * B], F32, tag="maskt")
            for b in range(B):
                for j in range(4):
                    nc.sync.dma_start(
                        mask_t[:, b * 4 + j: b * 4 + j + 1],
                        d["obs_mask"].ap()[b, j * 128:(j + 1) * 128])

            # ---- obs encoder (all batches), feature-major then transposed L3
            obsf = {}
            for b in range(B):
                oin = wkp.tile([36, NOBS], F32R, tag="oin")
                nc.sync.dma_start(oin[:], d["obs_in"].ap()[b])
                ps = pmlp.tile([128, NOBS], F32, tag="mlp")
                nc.tensor.matmul(ps[:], wo1t[:], oin[:], start=True, stop=True)
                of1 = wkp.tile([128, NOBS], F32R, tag="of1")
                act(of1[:], ps[:], AF.Gelu, bias=bo1t[:, 0:1])
                ps = pmlp.tile([128, NOBS], F32, tag="mlp")
                nc.tensor.matmul(ps[:], wo2t[:], of1[:], start=True, stop=True)
                of2 = wkp.tile([128, NOBS], F32R, tag="of2")
                act(of2[:], ps[:], AF.Gelu, bias=bo2t[:, 0:1])
                # L3 with transposed output: out[obs_chunk, cond]
                for j in range(4):
                    pst = pmlp.tile([128, COND], F32, tag="mlp")
                    nc.tensor.matmul(pst[:], of2[:, j * 128:(j + 1) * 128],
                                     wo3t[:], start=True, stop=False)
                    nc.tensor.matmul(pst[:], ones1[:], bo3r[:],
                                     start=False, stop=True)
                    oft = cp.tile([128, COND], F32R, tag=f"obsf{b}_{j}")
                    nc.vector.tensor_scalar_mul(
                        oft[:], pst[:], mask_t[:, b * 4 + j: b * 4 + j + 1])
                    obsf[(b, j)] = oft

            # ---- global feature MLP + head-bias fold (tiny) ----
            mean_t = []
            for k in range(2):
                mt = cp.tile([128, B], F32R, tag=f"mean{k}")
                nc.sync.dma_start(mt[:], d["mean_in"].ap()[k])
                mean_t.append(mt)
            g1 = []
            for m in range(2):
                ps = pmlp.tile([128, B], F32, tag="mlp")
                for k in range(2):
                    nc.tensor.matmul(ps[:], wg1s(k, m), mean_t[k][:],
                                     start=(k == 0), stop=(k == 1))
                t = cp.tile([128, B], F32R, tag=f"g1{m}")
                act(t[:], ps[:], AF.Gelu, bias=bg1t[:, m:m + 1])
                g1.append(t)
            g2 = []
            for m in range(2):
                ps = pmlp.tile([128, B], F32, tag="mlp")
                for k in range(2):
                    nc.tensor.matmul(ps[:], wg2s(k, m), g1[k][:],
                                     start=(k == 0), stop=(k == 1))
                t = cp.tile([128, B], F32R, tag=f"g2{m}")
                act(t[:], ps[:], AF.Identity, bias=bg2t[:, m:m + 1])
                g2.append(t)
            gb = []
            for m in range(2):
                ps = pmlp.tile([128, B], F32, tag="mlp")
                for k in range(2):
                    nc.tensor.matmul(ps[:], wh1gs(k, m), g2[k][:],
                                     start=(k == 0), stop=(k == 1))
                t = cp.tile([128, B], F32, tag=f"gb{m}")
                act(t[:], ps[:], AF.Identity, bias=bh1t[:, m:m + 1])
                gb.append(t)
            end_pass()   # preamble: gelu set

            # ---- main per-batch loop: A (gelu) -> B (exp) -> C (gelu) ----
            for _ in range(rep):
                for b in range(B):
                    pfb = bigp.tile([128, 2 * PPC], F32R, tag="pfb")
                    lcb = bigp.tile([128, PPC], F32R, tag="lcb")

                    # pass A: point encoder
                    for s in range(NCH):
                        sl = slice(s * CH, (s + 1) * CH)
                        infm = wkp.tile([8, CH], F32R, tag="infm")
                        nc.sync.dma_start(infm[:], d["xin"].ap()[b, :, sl])
                        h1 = [wkp.tile([128, CH], F32R, tag=f"h1{m}", name=f"h1{m}")
                              for m in range(2)]
                        for m in range(2):
                            ps = pmlp.tile([128, CH], F32, tag="mlp")
                            nc.tensor.matmul(ps[:], wp1s(0, m), infm[:],
                                             start=True, stop=True)
                            act(h1[m][:], ps[:], AF.Gelu, bias=bp1t[:, m:m + 1])
                        h2 = [wkp.tile([128, CH], F32R, tag=f"h2{m}", name=f"h2{m}")
                              for m in range(2)]
                        for m in range(2):
                            ps = pmlp.tile([128, CH], F32, tag="mlp")
                            for k in range(2):
                                nc.tensor.matmul(ps[:], wp2s(k, m), h1[k][:],
                                                 start=(k == 0), stop=(k == 1))
                            act(h2[m][:], ps[:], AF.Gelu, bias=bp2t[:, m:m + 1])
                        for m in range(2):
                            ps = pmlp.tile([128, CH], F32, tag="mlp")
                            for k in range(2):
                                nc.tensor.matmul(ps[:], wp3s(k, m), h2[k][:],
                                                 start=(k == 0), stop=(k == 1))
                            nc.vector.tensor_scalar_add(
                                pfb[:, m * PPC + s * CH: m * PPC + (s + 1) * CH],
                                ps[:], bp3t[:, m:m + 1])
                    end_pass()

                    # pass B: RBF softmax aggregation (d2 via one bf16
                    # matmul on hi/mid/lo coordinate splits; the bf16 qq row
                    # is a per-point shift that cancels in the softmax)
                    for s in range(NCH):
                        sl = slice(s * CH, (s + 1) * CH)
                        rhs = wkp.tile([22, CH], BF16, tag="rhsd2")
                        nc.sync.dma_start(rhs[:], d["cq_bf"].ap()[b, :, sl])

                        s_ps = pagg.tile([128, CH], F32, tag="s_ps")
                        lc_ps = pagg.tile([128, CH], F32, tag="lc_ps")
                        for j in range(4):
                            dps = pd2.tile([128, CH], F32, tag="d2")
                            nc.tensor.matmul(
                                dps[:],
                                obs_lhst_t[b][:, j * 128:(j + 1) * 128],
                                rhs[:], start=True, stop=True)
                            e = wkp.tile([128, CH], F32R, tag="e")
                            act(e[:], dps[:], AF.Exp, scale=SCALE)
                            nc.tensor.matmul(s_ps[:], ones128[:], e[:],
                                             start=(j == 0), stop=(j == 3))
                            nc.tensor.matmul(lc_ps[:], obsf[(b, j)][:], e[:],
                                             start=(j == 0), stop=(j == 3))
                        rec = wkp.tile([128, CH], F32, tag="rec")
                        nc.vector.reciprocal(rec[:], s_ps[:])
                        nc.vector.tensor_mul(out=lcb[:, sl], in0=lc_ps[:],
                                             in1=rec[:])
                    end_pass()

                    # pass C: head
                    for s in range(NCH):
                        sl = slice(s * CH, (s + 1) * CH)
                        hc1 = [wkp.tile([128, CH], F32R, tag=f"hc1{m}", name=f"hc1{m}")
                               for m in range(2)]
                        for m in range(2):
                            ps = pmlp.tile([128, CH], F32, tag="mlp")
                            for k in range(2):
                                nc.tensor.matmul(
                                    ps[:], wh1pfs(k, m),
                                    pfb[:, k * PPC + s * CH: k * PPC + (s + 1) * CH],
                                    start=(k == 0), stop=False)
                            nc.tensor.matmul(ps[:], wh1lcs(0, m), lcb[:, sl],
                                             start=False, stop=True)
                            act(hc1[m][:], ps[:], AF.Gelu,
                                bias=gb[m][:, b:b + 1])
                        hc2 = [wkp.tile([128, CH], F32R, tag=f"hc2{m}", name=f"hc2{m}")
                               for m in range(2)]
                        for m in range(2):
                            ps = pmlp.tile([128, CH], F32, tag="mlp")
                            for k in range(2):
                                nc.tensor.matmul(ps[:], wh2s(k, m), hc1[k][:],
                                                 start=(k == 0), stop=(k == 1))
                            act(hc2[m][:], ps[:], AF.Gelu,
                                bias=bh2t[:, m:m + 1])
                        ps = pmlp.tile([NF, CH], F32, tag="mlp")
                        for k in range(2):
                            nc.tensor.matmul(ps[:], wh3t[:, k * NF:(k + 1) * NF],
                                             hc2[k][:],
                                             start=(k == 0), stop=(k == 1))
                        osb = wkp.tile([NF, CH], F32, tag="osb")
                        nc.vector.tensor_scalar_add(osb[:], ps[:],
                                                    bh3t[:, 0:1])
                        nc.sync.dma_start(y.ap()[b, :, sl], osb[:])
                    end_pass()

            # enforce ACT engine-order across passes so walrus doesn't thrash
            # the activation table sets (gelu vs exp live in different sets)
            for i in range(1, len(acts)):
                add_dep_helper(acts[i][0].ins, acts[i - 1][-1].ins,
                               reason="ACT table-set ordering")
    nc.compile()
    return nc


# -------------------- host side --------------------

_CACHE = {}


def _get(name, rep=1):
    key = (name, rep)
    if key not in _CACHE:
        _CACHE[key] = (build_launch1(rep) if name == "l1" else
                       build_launch2(rep))
    return _CACHE[key]


def _prep(t, x_t, coords, obs_coords, obs_values, obs_mask, obs_field_ids,
          params):
    """Build the per-core input maps (host-side layout prep only)."""
    f32 = np.float32
    t = np.asarray(t, f32)
    x_t = np.asarray(x_t, f32)
    coords = np.asarray(coords, f32)
    obs_coords = np.asarray(obs_coords, f32)
    obs_values = np.asarray(obs_values, f32)
    obs_mask = np.asarray(obs_mask, f32)
    ids = np.asarray(obs_field_ids)

    import ml_dtypes
    bf16 = ml_dtypes.bfloat16

    def split3(x):
        xh = x.astype(bf16)
        xm = (x - xh.astype(f32)).astype(bf16)
        xl = (x - xh.astype(f32) - xm.astype(f32)).astype(bf16)
        return xh, xm, xl

    coords_t = np.ascontiguousarray(coords.transpose(0, 2, 1))       # [B,3,N]
    xin = np.concatenate([
        coords_t,
        np.ascontiguousarray(x_t.transpose(0, 2, 1)),
        np.broadcast_to(t[:, None, None], (B, 1, NPTS)),
    ], axis=1).astype(f32)                                           # [B,8,N]

    fe = np.asarray(params["field_embed"], f32)[np.clip(ids, 0, None)]
    fe = fe * obs_mask[..., None]                                    # [B,M,32]
    obs_in = np.concatenate([obs_coords, obs_values, fe], axis=-1)   # [B,M,36]
    obs_in = np.ascontiguousarray(obs_in.transpose(0, 2, 1)).astype(f32)

    # hi/mid/lo bf16 decomposition for the single-matmul d2
    # pairs: (oh,qh),(oh,qm),(oh,ql),(om,qh),(om,qm),(ol,qh) per coord,
    # then oo hi/mid/lo vs ones, then ones vs qq (qq row written on device)
    oo = (obs_coords * obs_coords).sum(-1).astype(f32)               # [B,M]
    ot = obs_coords.transpose(0, 2, 1)                               # [B,3,M]
    oh, om, ol = split3(ot)
    ooh, oom, ool = split3(oo)
    n2 = lambda a: (-2.0 * a.astype(f32)).astype(bf16)
    obs_lhst = np.concatenate([
        n2(oh), n2(oh), n2(oh), n2(om), n2(om), n2(ol),
        (ooh.astype(f32) + (1.0 - obs_mask) * np.float32(1e6)
         ).astype(bf16)[:, None, :],
        oom[:, None, :], ool[:, None, :],
        np.ones((B, 1, NOBS), bf16),
    ], axis=1)                                                       # [B,22,M]
    qh, qm, ql = split3(coords_t)
    qq = (coords_t * coords_t).sum(1, dtype=f32)                     # [B,N]
    cq_bf = np.concatenate([
        qh, qm, ql, qh, qm, qh,
        np.ones((B, 3, NPTS), bf16),
        qq.astype(bf16)[:, None, :],
    ], axis=1)                                                       # [B,22,N]

    def wb(layers, i):
        w, bias = layers[i]
        return np.ascontiguousarray(np.asarray(w, f32)), \
            np.ascontiguousarray(np.asarray(bias, f32))

    pe = params["point_encoder"]
    oe = params["obs_encoder"]
    ge = params["global_encoder"]
    hd = params["head"]
    wp1, bp1 = wb(pe, 0); wp2, bp2 = wb(pe, 1); wp3, bp3 = wb(pe, 2)
    wo1, bo1 = wb(oe, 0); wo2, bo2 = wb(oe, 1); wo3, bo3 = wb(oe, 2)
    wg1, bg1 = wb(ge, 0); wg2, bg2 = wb(ge, 1)
    wh1, bh1 = wb(hd, 0); wh2, bh2 = wb(hd, 1); wh3, bh3 = wb(hd, 2)

    shared2 = dict(
        obs_in=obs_in, obs_lhst=obs_lhst, obs_mask=obs_mask,
        wp1=wp1, wp2=wp2, wp3=wp3, bp1=bp1, bp2=bp2, bp3=bp3,
        wo1=wo1, wo2=wo2, wo3=wo3, bo1=bo1, bo2=bo2,
        bo3r=np.ascontiguousarray(bo3[None, :]),
        wg1=wg1, wg2=wg2, bg1=bg1, bg2=bg2,
        wh1pf=np.ascontiguousarray(wh1[0:HID]),
        wh1g=np.ascontiguousarray(wh1[HID:2 * HID]),
        wh1lc=np.ascontiguousarray(wh1[2 * HID:]),
        bh1=bh1, wh2=wh2, bh2=bh2, wh3=wh3, bh3=bh3,
        ones1=np.ones((1, 128), np.float32),
        ones128=np.ones((128, 128), np.float32),
    )
    in1, in2 = [], []
    for c in range(N_CORES):
        sl = slice(c * PPC, (c + 1) * PPC)
        xin_c = np.ascontiguousarray(xin[:, :, sl])
        in1.append(dict(xin=xin_c, wp1=wp1, wp2=wp2, wp3=wp3,
                        bp1=bp1, bp2=bp2, bp3=bp3))
        m = dict(shared2)
        m["xin"] = xin_c
        m["cq_bf"] = np.ascontiguousarray(cq_bf[:, :, sl])
        in2.append(m)
    return in1, in2


def _run_l1(in1, rep=1):
    nc = _get("l1", rep)
    res = bass_utils.run_bass_kernel_spmd(nc, in1,
                                          core_ids=list(range(N_CORES)))
    # acc [128, 2B] per core; combine in f64
    total = np.zeros((128, 2 * B), np.float64)
    for r in res.results:
        total += r["acc"].astype(np.float64)
    mean = total / float(NPTS)
    # mean_in [2, 128, B]: mean_in[k, f, b] = mean[f, b*2+k]
    mean_in = np.empty((2, 128, B), np.float32)
    for k in range(2):
        for b in range(B):
            mean_in[k, :, b] = mean[:, b * 2 + k].astype(np.float32)
    return mean_in


def _run_l2(in2, mean_in, rep=1):
    nc = _get("l2", rep)
    for m in in2:
        m["mean_in"] = mean_in
    res = bass_utils.run_bass_kernel_spmd(nc, in2,
                                          core_ids=list(range(N_CORES)))
    out = np.empty((B, NPTS, NF), np.float32)
    for c, r in enumerate(res.results):
        yc = r["y"]                                   # [B, NF, PPC]
        out[:, c * PPC:(c + 1) * PPC, :] = yc.transpose(0, 2, 1)
    return out


def kernel(t, x_t, coords, obs_coords, obs_values, obs_mask, obs_field_ids,
           params):
    in1, in2 = _prep(t, x_t, coords, obs_coords, obs_values, obs_mask,
                     obs_field_ids, params)
    mean_in = _run_l1(in1)
    return _run_l2(in2, mean_in)


# revision 13
# speedup vs baseline: 1.2949x; 1.1321x over previous
"""ConditionalPointFFM fused Bass kernel for 8 TRN2 NeuronCores.

Model (B=4 batches, N=32768 query points, M=512 obs points):
  point_feat = MLP_p(concat[coords, x_t, t])                  [B,N,256]
  obs_feat   = MLP_o(concat[obs_coords, obs_values, fe]) * mask
  d2         = |q - o|^2 (masked)  ->  w = softmax(-d2/(2s^2))
  local_cond = w @ obs_feat                                   [B,N,128]
  g          = MLP_g(point_feat.mean(axis=1))
  out        = MLP_h(concat[point_feat, g, local_cond])       [B,N,4]

Sharding: every core holds all 4 batches and a contiguous 4096-point
slice of N. Obs tensors + params replicated. The mean over N needs an
all-reduce; device collectives are unavailable here, so we run two
launches and combine the 8 per-core partial sums on the host between
them (the tiny global MLP still runs on device in launch 2).

Numerics: MLP matmuls run as float32r (full PE rate, ~1.7e-4); the d2
matmul runs true fp32 (f32r's rounding is amplified x200 by the RBF
exp). exp(-200*d2) needs no max-subtraction: d2min <= 0.435 with
overwhelming probability, so the unnormalized weights stay in fp32
range; the denominator comes from a ones-matrix matmul which directly
yields a partition-broadcast sum.

Layout: all activations feature-major [feat(partition), points(free)],
so y = x @ W becomes matmul(lhsT=W, rhs=x_fm) and the per-feature bias
is the ACT instruction's per-partition bias operand. d2 is produced
obs-major by a single K=5 matmul: lhsT rows [-2*o, oo + (1-mask)*1e6, 1],
rhs rows [q, 1, qq].
"""

import numpy as np

import concourse.bacc as bacc
import concourse.tile as tile
import concourse.mybir as mybir
import concourse.bass_isa as bass_isa
from concourse.tile_rust import add_dep_helper
from concourse import bass_utils

N_CORES = 8
B = 4
NPTS = 32768
PPC = NPTS // N_CORES          # points per core per batch
CH = 512                       # chunk of points processed at once
NCH = PPC // CH                # chunks per batch per core
NOBS = 512
HID = 256
COND = 128
FED = 32
NF = 4
SCALE = float(np.float32(-1.0 / (2 * 0.05 ** 2 + 1e-12)))   # == -200.0

F32 = mybir.dt.float32
F32R = mybir.dt.float32r
BF16 = mybir.dt.bfloat16
AF = mybir.ActivationFunctionType


def _load_w(nc, pool, name, dram, kin, kout):
    """DRAM weight [kin, kout] -> sbuf tile [128, (kin/128)*kout] in k-chunks,
    f32r. Returns (tile, slicer) where slicer(k, m) gives the [128,128] (or
    [kin,128]) lhsT AP for contraction chunk k, output chunk m."""
    nkin = max(1, kin // 128)
    pdim = min(kin, 128)
    t = pool.tile([pdim, nkin * kout], F32R, tag=name)
    for k in range(nkin):
        nc.sync.dma_start(t[:, k * kout:(k + 1) * kout],
                          dram.ap()[k * 128:k * 128 + pdim, :])

    def sl(k, m):
        return t[:, k * kout + m * 128: k * kout + m * 128 + 128]
    return t, sl


def _load_b(nc, pool, name, dram, n):
    """DRAM bias [n] -> sbuf [128, n/128] fp32 column tiles."""
    ncol = max(1, n // 128)
    pdim = min(n, 128)
    t = pool.tile([pdim, ncol], F32, tag=name)
    for m in range(ncol):
        nc.sync.dma_start(t[:, m:m + 1], dram.ap()[m * 128:m * 128 + pdim])
    return t


def build_launch1(rep=1):
    """Point-encoder partial sums: out acc [128, 8] (feat, b*2+ftile)."""
    nc = bacc.Bacc("TRN2", target_bir_lowering=False, debug=False,
                   num_devices=N_CORES)
    xin = nc.dram_tensor("xin", [B, 8, PPC], F32R, kind="ExternalInput")
    wp1 = nc.dram_tensor("wp1", [8, HID], F32R, kind="ExternalInput")
    wp2 = nc.dram_tensor("wp2", [HID, HID], F32R, kind="ExternalInput")
    wp3 = nc.dram_tensor("wp3", [HID, HID], F32R, kind="ExternalInput")
    bp1 = nc.dram_tensor("bp1", [HID], F32, kind="ExternalInput")
    bp2 = nc.dram_tensor("bp2", [HID], F32, kind="ExternalInput")
    bp3 = nc.dram_tensor("bp3", [HID], F32, kind="ExternalInput")
    acc_out = nc.dram_tensor("acc", [128, 2 * B], F32, kind="ExternalOutput")
    SC = 2 * CH
    NSC = PPC // SC

    with tile.TileContext(nc) as tc:
        with (
            tc.tile_pool(name="const", bufs=1) as cp,
            tc.tile_pool(name="work", bufs=3) as wp,
            tc.tile_pool(name="ps", bufs=2, space="PSUM") as pp,
        ):
            _, wp1s = _load_w(nc, cp, "wp1", wp1, 8, HID)
            _, wp2s = _load_w(nc, cp, "wp2", wp2, HID, HID)
            _, wp3s = _load_w(nc, cp, "wp3", wp3, HID, HID)
            bp1t = _load_b(nc, cp, "bp1", bp1, HID)
            bp2t = _load_b(nc, cp, "bp2", bp2, HID)
            bp3t = _load_b(nc, cp, "bp3", bp3, HID)
            acc = cp.tile([128, 2 * B], F32, tag="accsb")
            nc.vector.memset(acc[:], 0.0)

            for _ in range(rep):
                for b in range(B):
                    for s in range(NSC):
                        sl = slice(s * SC, (s + 1) * SC)
                        infm = wp.tile([8, SC], F32R, tag="infm")
                        nc.sync.dma_start(infm[:], xin.ap()[b, :, sl])
                        h1 = [wp.tile([128, SC], F32R, tag=f"h1{m}",
                                      name=f"h1{m}") for m in range(2)]
                        for m in range(2):
                            ps = pp.tile([128, SC], F32, tag="mlp")
                            for h in range(2):
                                hs = slice(h * CH, (h + 1) * CH)
                                nc.tensor.matmul(ps[:, hs], wp1s(0, m),
                                                 infm[:, hs],
                                                 start=True, stop=True)
                            nc.scalar.activation(h1[m][:], ps[:], AF.Gelu,
                                                 bias=bp1t[:, m:m + 1])
                        h2 = [wp.tile([128, SC], F32R, tag=f"h2{m}",
                                      name=f"h2{m}") for m in range(2)]
                        for m in range(2):
                            ps = pp.tile([128, SC], F32, tag="mlp")
                            for h in range(2):
                                hs = slice(h * CH, (h + 1) * CH)
                                for k in range(2):
                                    nc.tensor.matmul(ps[:, hs], wp2s(k, m),
                                                     h1[k][:, hs],
                                                     start=(k == 0),
                                                     stop=(k == 1))
                            nc.scalar.activation(h2[m][:], ps[:], AF.Gelu,
                                                 bias=bp2t[:, m:m + 1])
                        for m in range(2):
                            ps = pp.tile([128, SC], F32, tag="mlp")
                            for h in range(2):
                                hs = slice(h * CH, (h + 1) * CH)
                                for k in range(2):
                                    nc.tensor.matmul(ps[:, hs], wp3s(k, m),
                                                     h2[k][:, hs],
                                                     start=(k == 0),
                                                     stop=(k == 1))
                            pf = wp.tile([128, SC], F32, tag="pf")
                            red = wp.tile([128, 1], F32, tag="red")
                            nc.vector.tensor_scalar_add(pf[:], ps[:],
                                                        bp3t[:, m:m + 1])
                            nc.vector.reduce_sum(out=red[:], in_=pf[:],
                                                 axis=mybir.AxisListType.X)
                            col = b * 2 + m
                            nc.vector.tensor_add(out=acc[:, col:col + 1],
                                                 in0=acc[:, col:col + 1],
                                                 in1=red[:])
            nc.sync.dma_start(acc_out.ap(), acc[:])
    nc.compile()
    return nc


def build_launch2(rep=1):
    nc = bacc.Bacc("TRN2", target_bir_lowering=False, debug=False,
                   num_devices=N_CORES)
    dt_in = [
        ("xin", [B, 8, PPC], F32R),
        ("cq_bf", [B, 22, PPC], BF16),
        ("obs_lhst", [B, 22, NOBS], BF16), ("obs_in", [B, 36, NOBS], F32R),
        ("obs_mask", [B, NOBS], F32), ("mean_in", [2, 128, B], F32R),
        ("wp1", [8, HID], F32R), ("wp2", [HID, HID], F32R),
        ("wp3", [HID, HID], F32R),
        ("bp1", [HID], F32), ("bp2", [HID], F32), ("bp3", [HID], F32),
        ("wo1", [36, COND], F32R), ("wo2", [COND, COND], F32R),
        ("wo3", [COND, COND], F32R),
        ("bo1", [COND], F32), ("bo2", [COND], F32), ("bo3r", [1, COND], F32R),
        ("wg1", [HID, HID], F32R), ("wg2", [HID, HID], F32R),
        ("bg1", [HID], F32), ("bg2", [HID], F32),
        ("wh1pf", [HID, HID], F32R), ("wh1g", [HID, HID], F32R),
        ("wh1lc", [COND, HID], F32R), ("bh1", [HID], F32),
        ("wh2", [HID, HID], F32R), ("bh2", [HID], F32),
        ("wh3", [HID, NF], F32R), ("bh3", [NF], F32),
        ("ones1", [1, 128], F32R), ("ones128", [128, 128], F32R),
    ]
    d = {name: nc.dram_tensor(name, shp, dt, kind="ExternalInput")
         for name, shp, dt in dt_in}
    y = nc.dram_tensor("y", [B, NF, PPC], F32, kind="ExternalOutput")

    SC = 2 * CH                  # superchunk: 1024 points
    NSC = PPC // SC
    BG = 1                       # batches per pass-group

    acts = []
    cur_pass = []

    def end_pass():
        nonlocal cur_pass
        if cur_pass:
            acts.append(cur_pass)
            cur_pass = []

    def act(*args, **kw):
        inst = nc.scalar.activation(*args, **kw)
        cur_pass.append(inst)
        return inst

    with tile.TileContext(nc) as tc:
        with (
            tc.tile_pool(name="const", bufs=1) as cp,
            tc.tile_pool(name="big", bufs=1) as bigp,
            tc.tile_pool(name="work", bufs=2) as wkp,
            tc.tile_pool(name="ps", bufs=1, space="PSUM") as pp,
        ):
            # ---- constants ----
            _, wp1s = _load_w(nc, cp, "wp1", d["wp1"], 8, HID)
            _, wp2s = _load_w(nc, cp, "wp2", d["wp2"], HID, HID)
            _, wp3s = _load_w(nc, cp, "wp3", d["wp3"], HID, HID)
            bp1t = _load_b(nc, cp, "bp1", d["bp1"], HID)
            bp2t = _load_b(nc, cp, "bp2", d["bp2"], HID)
            bp3t = _load_b(nc, cp, "bp3", d["bp3"], HID)
            wo1t, _ = _load_w(nc, cp, "wo1", d["wo1"], 36, COND)
            wo2t, _ = _load_w(nc, cp, "wo2", d["wo2"], COND, COND)
            wo3t, _ = _load_w(nc, cp, "wo3", d["wo3"], COND, COND)
            bo1t = _load_b(nc, cp, "bo1", d["bo1"], COND)
            bo2t = _load_b(nc, cp, "bo2", d["bo2"], COND)
            bo3r = cp.tile([1, COND], F32R, tag="bo3r")
            nc.sync.dma_start(bo3r[:], d["bo3r"].ap())
            _, wg1s = _load_w(nc, cp, "wg1", d["wg1"], HID, HID)
            _, wg2s = _load_w(nc, cp, "wg2", d["wg2"], HID, HID)
            bg1t = _load_b(nc, cp, "bg1", d["bg1"], HID)
            bg2t = _load_b(nc, cp, "bg2", d["bg2"], HID)
            _, wh1pfs = _load_w(nc, cp, "wh1pf", d["wh1pf"], HID, HID)
            _, wh1gs = _load_w(nc, cp, "wh1g", d["wh1g"], HID, HID)
            _, wh1lcs = _load_w(nc, cp, "wh1lc", d["wh1lc"], COND, HID)
            bh1t = _load_b(nc, cp, "bh1", d["bh1"], HID)
            _, wh2s = _load_w(nc, cp, "wh2", d["wh2"], HID, HID)
            bh2t = _load_b(nc, cp, "bh2", d["bh2"], HID)
            wh3t, _ = _load_w(nc, cp, "wh3", d["wh3"], HID, NF)
            bh3t = _load_b(nc, cp, "bh3", d["bh3"], NF)
            ones1 = cp.tile([1, 128], F32R, tag="ones1")
            nc.sync.dma_start(ones1[:], d["ones1"].ap())
            ones128 = cp.tile([128, 128], F32R, tag="ones128")
            nc.sync.dma_start(ones128[:], d["ones128"].ap())

            obs_lhst_t = {}
            for b in range(B):
                t = cp.tile([22, NOBS], BF16, tag=f"obslhst{b}",
                            name=f"obslhst{b}")
                nc.sync.dma_start(t[:], d["obs_lhst"].ap()[b])
                obs_lhst_t[b] = t
            mask_t = cp.tile([128, 4 * B], F32, tag="maskt")
            for b in range(B):
                for j in range(4):
                    nc.sync.dma_start(
                        mask_t[:, b * 4 + j: b * 4 + j + 1],
                        d["obs_mask"].ap()[b, j * 128:(j + 1) * 128])

            # ---- obs encoder ----
            obsf = {}
            for b in range(B):
                oin = wkp.tile([36, NOBS], F32R, tag="oin")
                nc.sync.dma_start(oin[:], d["obs_in"].ap()[b])
                ps = pp.tile([128, 1024], F32, tag="mlp", bufs=3, name="mlp_ps")[:, 0:NOBS]
                nc.tensor.matmul(ps[:], wo1t[:], oin[:], start=True, stop=True)
                of1 = wkp.tile([128, NOBS], F32R, tag="of1")
                act(of1[:], ps[:], AF.Gelu, bias=bo1t[:, 0:1])
                ps = pp.tile([128, 1024], F32, tag="mlp", bufs=3, name="mlp_ps")[:, 0:NOBS]
                nc.tensor.matmul(ps[:], wo2t[:], of1[:], start=True, stop=True)
                of2 = wkp.tile([128, NOBS], F32R, tag="of2")
                act(of2[:], ps[:], AF.Gelu, bias=bo2t[:, 0:1])
                for j in range(4):
                    pst = pp.tile([128, 1024], F32, tag="mlp", bufs=3, name="mlp_ps")[:, 0:COND]
                    nc.tensor.matmul(pst[:], of2[:, j * 128:(j + 1) * 128],
                                     wo3t[:], start=True, stop=False)
                    nc.tensor.matmul(pst[:], ones1[:], bo3r[:],
                                     start=False, stop=True)
                    oft = cp.tile([128, COND], F32R, tag=f"obsf{b}_{j}",
                                  name=f"obsf{b}_{j}")
                    nc.vector.tensor_scalar_mul(
                        oft[:], pst[:], mask_t[:, b * 4 + j: b * 4 + j + 1])
                    obsf[(b, j)] = oft

            # ---- global feature MLP + head-bias fold ----
            mean_t = []
            for k in range(2):
                mt = cp.tile([128, B], F32R, tag=f"mean{k}", name=f"mean{k}")
                nc.sync.dma_start(mt[:], d["mean_in"].ap()[k])
                mean_t.append(mt)
            g1 = []
            for m in range(2):
                ps = pp.tile([128, 1024], F32, tag="mlp", bufs=3, name="mlp_ps")[:, 0:B]
                for k in range(2):
                    nc.tensor.matmul(ps[:], wg1s(k, m), mean_t[k][:],
                                     start=(k == 0), stop=(k == 1))
                t = cp.tile([128, B], F32R, tag=f"g1{m}", name=f"g1{m}")
                act(t[:], ps[:], AF.Gelu, bias=bg1t[:, m:m + 1])
                g1.append(t)
            g2 = []
            for m in range(2):
                ps = pp.tile([128, 1024], F32, tag="mlp", bufs=3, name="mlp_ps")[:, 0:B]
                for k in range(2):
                    nc.tensor.matmul(ps[:], wg2s(k, m), g1[k][:],
                                     start=(k == 0), stop=(k == 1))
                t = cp.tile([128, B], F32R, tag=f"g2{m}", name=f"g2{m}")
                act(t[:], ps[:], AF.Identity, bias=bg2t[:, m:m + 1])
                g2.append(t)
            gb = []
            for m in range(2):
                ps = pp.tile([128, 1024], F32, tag="mlp", bufs=3, name="mlp_ps")[:, 0:B]
                for k in range(2):
                    nc.tensor.matmul(ps[:], wh1gs(k, m), g2[k][:],
                                     start=(k == 0), stop=(k == 1))
                t = cp.tile([128, B], F32, tag=f"gb{m}", name=f"gb{m}")
                act(t[:], ps[:], AF.Identity, bias=bh1t[:, m:m + 1])
                gb.append(t)
            end_pass()   # preamble: gelu set

            for _ in range(rep):
                for g0 in range(0, B, BG):
                    grp = range(g0, g0 + BG)
                    pfb = {b: bigp.tile([128, 2 * PPC], F32R, tag=f"pfb{b - g0}",
                                        name=f"pfb{b - g0}") for b in grp}
                    lcb = {b: bigp.tile([128, PPC], F32R, tag=f"lcb{b - g0}",
                                        name=f"lcb{b - g0}") for b in grp}

                    # pass A: point encoder
                    for b in grp:
                        for s in range(NSC):
                            sl = slice(s * SC, (s + 1) * SC)
                            infm = wkp.tile([8, SC], F32R, tag="infm")
                            nc.sync.dma_start(infm[:], d["xin"].ap()[b, :, sl])
                            h1 = [wkp.tile([128, SC], F32R, tag=f"h1{m}",
                                           name=f"h1{m}") for m in range(2)]
                            for m in range(2):
                                ps = pp.tile([128, 1024], F32, tag="mlp", bufs=3, name="mlp_ps")
                                for h in range(2):
                                    hs = slice(h * CH, (h + 1) * CH)
                                    nc.tensor.matmul(ps[:, hs], wp1s(0, m),
                                                     infm[:, hs],
                                                     start=True, stop=True)
                                act(h1[m][:], ps[:], AF.Gelu,
                                    bias=bp1t[:, m:m + 1])
                            h2 = [wkp.tile([128, SC], F32R, tag=f"h2{m}",
                                           name=f"h2{m}") for m in range(2)]
                            for m in range(2):
                                ps = pp.tile([128, 1024], F32, tag="mlp", bufs=3, name="mlp_ps")
                                for h in range(2):
                                    hs = slice(h * CH, (h + 1) * CH)
                                    for k in range(2):
                                        nc.tensor.matmul(ps[:, hs], wp2s(k, m),
                                                         h1[k][:, hs],
                                                         start=(k == 0),
                                                         stop=(k == 1))
                                act(h2[m][:], ps[:], AF.Gelu,
                                    bias=bp2t[:, m:m + 1])
                            for m in range(2):
                                ps = pp.tile([128, 1024], F32, tag="mlp", bufs=3, name="mlp_ps")
                                for h in range(2):
                                    hs = slice(h * CH, (h + 1) * CH)
                                    for k in range(2):
                                        nc.tensor.matmul(ps[:, hs], wp3s(k, m),
                                                         h2[k][:, hs],
                                                         start=(k == 0),
                                                         stop=(k == 1))
                                nc.vector.tensor_scalar_add(
                                    pfb[b][:, m * PPC + s * SC:
                                           m * PPC + (s + 1) * SC],
                                    ps[:], bp3t[:, m:m + 1])
                    end_pass()

                    # pass B: RBF softmax aggregation
                    for b in grp:
                        for s in range(NSC):
                            rhs = wkp.tile([22, SC], BF16, tag="rhsd2")
                            nc.sync.dma_start(
                                rhs[:], d["cq_bf"].ap()[b, :,
                                                        s * SC:(s + 1) * SC])
                            for h in range(2):
                                hs = slice(h * CH, (h + 1) * CH)
                                s_ps = pp.tile([128, CH], F32, tag="s_ps", name="s_ps")
                                lc_ps = pp.tile([128, CH], F32, tag="lc_ps", name="lc_ps")
                                for j in range(4):
                                    dps = pp.tile([128, 1024], F32, tag="mlp", bufs=3, name="mlp_ps")[:, 0:CH]
                                    nc.tensor.matmul(
                                        dps[:],
                                        obs_lhst_t[b][:, j * 128:(j + 1) * 128],
                                        rhs[:, hs], start=True, stop=True)
                                    e = wkp.tile([128, CH], F32R, tag="e")
                                    act(e[:], dps[:], AF.Exp, scale=SCALE)
                                    nc.tensor.matmul(s_ps[:], ones128[:], e[:],
                                                     start=(j == 0),
                                                     stop=(j == 3))
                                    nc.tensor.matmul(lc_ps[:],
                                                     obsf[(b, j)][:], e[:],
                                                     start=(j == 0),
                                                     stop=(j == 3))
                                rec = wkp.tile([128, CH], F32, tag="rec")
                                nc.vector.reciprocal(rec[:], s_ps[:])
                                nc.vector.tensor_mul(
                                    out=lcb[b][:, s * SC + h * CH:
                                               s * SC + (h + 1) * CH],
                                    in0=lc_ps[:], in1=rec[:])
                    end_pass()

                    # pass C: head
                    for b in grp:
                        for s in range(NSC):
                            hc1 = [wkp.tile([128, SC], F32R, tag=f"hc1{m}",
                                            name=f"hc1{m}") for m in range(2)]
                            for m in range(2):
                                ps = pp.tile([128, 1024], F32, tag="mlp", bufs=3, name="mlp_ps")
                                for h in range(2):
                                    hs0 = s * SC + h * CH
                                    hs = slice(h * CH, (h + 1) * CH)
                                    for k in range(2):
                                        nc.tensor.matmul(
                                            ps[:, hs], wh1pfs(k, m),
                                            pfb[b][:, k * PPC + hs0:
                                                   k * PPC + hs0 + CH],
                                            start=(k == 0), stop=False)
                                    nc.tensor.matmul(
                                        ps[:, hs], wh1lcs(0, m),
                                        lcb[b][:, hs0:hs0 + CH],
                                        start=False, stop=True)
                                act(hc1[m][:], ps[:], AF.Gelu,
                                    bias=gb[m][:, b:b + 1])
                            hc2 = [wkp.tile([128, SC], F32R, tag=f"hc2{m}",
                                            name=f"hc2{m}") for m in range(2)]
                            for m in range(2):
                                ps = pp.tile([128, 1024], F32, tag="mlp", bufs=3, name="mlp_ps")
                                for h in range(2):
                                    hs = slice(h * CH, (h + 1) * CH)
                                    for k in range(2):
                                        nc.tensor.matmul(ps[:, hs], wh2s(k, m),
                                                         hc1[k][:, hs],
                                                         start=(k == 0),
                                                         stop=(k == 1))
                                act(hc2[m][:], ps[:], AF.Gelu,
                                    bias=bh2t[:, m:m + 1])
                            ps = pp.tile([128, 1024], F32, tag="mlp", bufs=3, name="mlp_ps")[0:NF, :]
                            for h in range(2):
                                hs = slice(h * CH, (h + 1) * CH)
                                for k in range(2):
                                    nc.tensor.matmul(
                                        ps[:, hs],
                                        wh3t[:, k * NF:(k + 1) * NF],
                                        hc2[k][:, hs],
                                        start=(k == 0), stop=(k == 1))
                            osb = wkp.tile([NF, SC], F32, tag="osb")
                            nc.vector.tensor_scalar_add(osb[:], ps[:],
                                                        bh3t[:, 0:1])
                            nc.sync.dma_start(
                                y.ap()[b, :, s * SC:(s + 1) * SC], osb[:])
                    end_pass()

            for i in range(1, len(acts)):
                add_dep_helper(acts[i][0].ins, acts[i - 1][-1].ins,
                               reason="ACT table-set ordering")
    nc.compile()
    return nc


# -------------------- host side --------------------

_CACHE = {}


def _get(name, rep=1):
    key = (name, rep)
    if key not in _CACHE:
        _CACHE[key] = (build_launch1(rep) if name == "l1" else
                       build_launch2(rep))
    return _CACHE[key]


def _prep(t, x_t, coords, obs_coords, obs_values, obs_mask, obs_field_ids,
          params):
    """Build the per-core input maps (host-side layout prep only)."""
    f32 = np.float32
    t = np.asarray(t, f32)
    x_t = np.asarray(x_t, f32)
    coords = np.asarray(coords, f32)
    obs_coords = np.asarray(obs_coords, f32)
    obs_values = np.asarray(obs_values, f32)
    obs_mask = np.asarray(obs_mask, f32)
    ids = np.asarray(obs_field_ids)

    import ml_dtypes
    bf16 = ml_dtypes.bfloat16

    def split3(x):
        xh = x.astype(bf16)
        xm = (x - xh.astype(f32)).astype(bf16)
        xl = (x - xh.astype(f32) - xm.astype(f32)).astype(bf16)
        return xh, xm, xl

    coords_t = np.ascontiguousarray(coords.transpose(0, 2, 1))       # [B,3,N]
    xin = np.concatenate([
        coords_t,
        np.ascontiguousarray(x_t.transpose(0, 2, 1)),
        np.broadcast_to(t[:, None, None], (B, 1, NPTS)),
    ], axis=1).astype(f32)                                           # [B,8,N]

    fe = np.asarray(params["field_embed"], f32)[np.clip(ids, 0, None)]
    fe = fe * obs_mask[..., None]                                    # [B,M,32]
    obs_in = np.concatenate([obs_coords, obs_values, fe], axis=-1)   # [B,M,36]
    obs_in = np.ascontiguousarray(obs_in.transpose(0, 2, 1)).astype(f32)

    # hi/mid/lo bf16 decomposition for the single-matmul d2
    # pairs: (oh,qh),(oh,qm),(oh,ql),(om,qh),(om,qm),(ol,qh) per coord,
    # then oo hi/mid/lo vs ones, then ones vs qq (qq row written on device)
    oo = (obs_coords * obs_coords).sum(-1).astype(f32)               # [B,M]
    ot = obs_coords.transpose(0, 2, 1)                               # [B,3,M]
    oh, om, ol = split3(ot)
    ooh, oom, ool = split3(oo)
    n2 = lambda a: (-2.0 * a.astype(f32)).astype(bf16)
    obs_lhst = np.concatenate([
        n2(oh), n2(oh), n2(oh), n2(om), n2(om), n2(ol),
        (ooh.astype(f32) + (1.0 - obs_mask) * np.float32(1e6)
         ).astype(bf16)[:, None, :],
        oom[:, None, :], ool[:, None, :],
        np.ones((B, 1, NOBS), bf16),
    ], axis=1)                                                       # [B,22,M]
    qh, qm, ql = split3(coords_t)
    qq = (coords_t * coords_t).sum(1, dtype=f32)                     # [B,N]
    cq_bf = np.concatenate([
        qh, qm, ql, qh, qm, qh,
        np.ones((B, 3, NPTS), bf16),
        qq.astype(bf16)[:, None, :],
    ], axis=1)                                                       # [B,22,N]

    def wb(layers, i):
        w, bias = layers[i]
        return np.ascontiguousarray(np.asarray(w, f32)), \
            np.ascontiguousarray(np.asarray(bias, f32))

    pe = params["point_encoder"]
    oe = params["obs_encoder"]
    ge = params["global_encoder"]
    hd = params["head"]
    wp1, bp1 = wb(pe, 0); wp2, bp2 = wb(pe, 1); wp3, bp3 = wb(pe, 2)
    wo1, bo1 = wb(oe, 0); wo2, bo2 = wb(oe, 1); wo3, bo3 = wb(oe, 2)
    wg1, bg1 = wb(ge, 0); wg2, bg2 = wb(ge, 1)
    wh1, bh1 = wb(hd, 0); wh2, bh2 = wb(hd, 1); wh3, bh3 = wb(hd, 2)

    shared2 = dict(
        obs_in=obs_in, obs_lhst=obs_lhst, obs_mask=obs_mask,
        wp1=wp1, wp2=wp2, wp3=wp3, bp1=bp1, bp2=bp2, bp3=bp3,
        wo1=wo1, wo2=wo2, wo3=wo3, bo1=bo1, bo2=bo2,
        bo3r=np.ascontiguousarray(bo3[None, :]),
        wg1=wg1, wg2=wg2, bg1=bg1, bg2=bg2,
        wh1pf=np.ascontiguousarray(wh1[0:HID]),
        wh1g=np.ascontiguousarray(wh1[HID:2 * HID]),
        wh1lc=np.ascontiguousarray(wh1[2 * HID:]),
        bh1=bh1, wh2=wh2, bh2=bh2, wh3=wh3, bh3=bh3,
        ones1=np.ones((1, 128), np.float32),
        ones128=np.ones((128, 128), np.float32),
    )
    in1, in2 = [], []
    for c in range(N_CORES):
        sl = slice(c * PPC, (c + 1) * PPC)
        xin_c = np.ascontiguousarray(xin[:, :, sl])
        in1.append(dict(xin=xin_c, wp1=wp1, wp2=wp2, wp3=wp3,
                        bp1=bp1, bp2=bp2, bp3=bp3))
        m = dict(shared2)
        m["xin"] = xin_c
        m["cq_bf"] = np.ascontiguousarray(cq_bf[:, :, sl])
        in2.append(m)
    return in1, in2


def _run_l1(in1, rep=1):
    nc = _get("l1", rep)
    res = bass_utils.run_bass_kernel_spmd(nc, in1,
                                          core_ids=list(range(N_CORES)))
    # acc [128, 2B] per core; combine in f64
    total = np.zeros((128, 2 * B), np.float64)
    for r in res.results:
        total += r["acc"].astype(np.float64)
    mean = total / float(NPTS)
    # mean_in [2, 128, B]: mean_in[k, f, b] = mean[f, b*2+k]
    mean_in = np.empty((2, 128, B), np.float32)
    for k in range(2):
        for b in range(B):
            mean_in[k, :, b] = mean[:, b * 2 + k].astype(np.float32)
    return mean_in


def _run_l2(in2, mean_in, rep=1):
    nc = _get("l2", rep)
    for m in in2:
        m["mean_in"] = mean_in
    res = bass_utils.run_bass_kernel_spmd(nc, in2,
                                          core_ids=list(range(N_CORES)))
    out = np.empty((B, NPTS, NF), np.float32)
    for c, r in enumerate(res.results):
        yc = r["y"]                                   # [B, NF, PPC]
        out[:, c * PPC:(c + 1) * PPC, :] = yc.transpose(0, 2, 1)
    return out


def kernel(t, x_t, coords, obs_coords, obs_values, obs_mask, obs_field_ids,
           params):
    in1, in2 = _prep(t, x_t, coords, obs_coords, obs_values, obs_mask,
                     obs_field_ids, params)
    mean_in = _run_l1(in1)
    return _run_l2(in2, mean_in)
